# revision 13
# baseline (speedup 1.0000x reference)
"""GCNConv (transform + symmetric-norm aggregate + sigmoid) on 8 Trainium2 NeuronCores.

out_i = sigmoid(dinv_i * sum_{j->i} dinv_j*(xW)_j + dinv_i^2*(xW)_i + b),
dinv = 1/sqrt(1 + in_degree).

Device algorithm (SPMD over 8 cores; per-core differences are pure data):
  pass A: g = (dinv*x) @ W in bf16 for all nodes on every core (dinv folded
          into x on the host; tiled matmul from a host-transposed x; g stored
          in a partition-tiled HBM layout of 256B rows — 64 bf16 payload + pad
          — split into 4 quarter-tables so gather indices fit in int16)
  pass B: per 128-dst-node tile: 4 dma_gather calls (one per quarter, spread
          over 4 SWDGE queues, single-packet, trailing -1 idx skip padding)
          pull g[src] rows for the tile's dst-bucketed edge list; one-hot S
          built in bf16 on DVE (is_equal of local-dst ids vs iota); segment
          sum via bf16 PE matmuls accumulated in f32 PSUM (self-loop chunk
          reads own rows captured during pass A); finalize per 7-tile group:
          dinv_dst * psum + b, sigmoid, one batched store.

Each core's inputs are rotated by its tile offset so the program is address-
uniform: core c sees global node-tile (t + c*nt_core) % nt_pad at position t,
and its own output tiles are always tiles [0, nt_core).

Host side only re-formats data: COO->CSR bucket sort, degree/dinv, padding,
int16 index encoding, x transpose + per-core rotation.
"""

import sys

for _p in ("/opt/trn_rl_repo", "/root/.axon_site/_ro/trn_rl_repo"):
    if _p not in sys.path:
        sys.path.append(_p)

import ml_dtypes
import numpy as np

import concourse.bacc as bacc
import concourse.bass as bass
import concourse.mybir as mybir
import concourse.tile as tile
from concourse.bass import ts
from concourse.bass_utils import run_bass_kernel_spmd

P = 128
N_CORES = 8
BATCH_A = 16  # node tiles per pass-A iteration
FIN_B = 7  # dst tiles per pass-B finalize group
NQ = 4  # quarter tables (int16 index range)
NQUEUES = 4  # SWDGE queues; quarter q -> queue q
GROW = 128  # g-table row width in bf16 elems (256B rows; first 64 = payload)
PAD_NEG1 = False  # encode gather padding slots as idx -1 (DGE skips trailing -1)

BF16 = ml_dtypes.bfloat16

_prog_cache: dict = {}


def _plan(n_nodes: int):
    nt_real = -(-n_nodes // P)
    nt_pad = nt_real
    while (
        nt_pad % N_CORES
        or (nt_pad // N_CORES) % FIN_B
        or nt_pad % BATCH_A
    ):
        nt_pad += 1
    return nt_real, nt_pad, nt_pad * P, nt_pad // N_CORES


def preprocess(x: np.ndarray, edge_index: np.ndarray, W: np.ndarray, b: np.ndarray):
    n_nodes, hid = x.shape
    out_dim = W.shape[1]
    nt_real, nt_pad, npad, nt_core = _plan(n_nodes)

    src = np.ascontiguousarray(edge_index[0]).astype(np.int64)
    dst = np.ascontiguousarray(edge_index[1]).astype(np.int64)
    e = src.shape[0]

    deg = np.bincount(dst, minlength=npad).astype(np.float64) + 1.0  # self-loop
    dinv_full = (1.0 / np.sqrt(deg)).astype(np.float32)  # [npad]

    # bucket edges by (dst tile, src quarter), stable
    tile_of = dst // P
    qr_of = (src % P) // 32
    order = np.argsort(tile_of * NQ + qr_of, kind="stable")
    src_s = src[order]
    dst_s = dst[order]
    grp_s = (tile_of * NQ + qr_of)[order]

    grp_counts = np.bincount(grp_s, minlength=nt_pad * NQ)
    jq = int(max(1, -(-int(grp_counts.max()) // P)))  # chunks per (tile, quarter)
    jc = NQ * jq + 1  # chunks per tile incl. own/self-loop chunk
    slot_cap = jq * P

    grp_start = np.zeros(nt_pad * NQ, dtype=np.int64)
    np.cumsum(grp_counts[:-1], out=grp_start[1:])
    pos = np.arange(e, dtype=np.int64) - grp_start[grp_s]
    slot = grp_s * slot_cap + pos

    # per-edge gather info (tile-rotation applied per core later)
    nslots = nt_pad * NQ * slot_cap
    loc_pp = np.zeros(nslots, dtype=np.int64)
    loc_tg = np.zeros(nslots, dtype=np.int64)
    valid = np.zeros(nslots, dtype=bool)
    dl_flat = np.full(nslots, -1.0, dtype=np.float32)
    loc_pp[slot] = (src_s % P) % 32
    loc_tg[slot] = src_s // P
    valid[slot] = True
    dl_flat[slot] = (dst_s - (dst_s // P) * P).astype(np.float32)

    loc_pp3 = loc_pp.reshape(nt_pad, NQ, slot_cap)
    loc_tg3 = loc_tg.reshape(nt_pad, NQ, slot_cap)
    valid3 = valid.reshape(nt_pad, NQ, slot_cap)

    # dl input [P, nt_pad, jc]: chunk cc=(qr*jq+j) at col t*jc+cc; own chunk last
    dl4 = dl_flat.reshape(nt_pad, NQ * jq, P)  # [t, cc, p]
    dl_all = np.empty((P, nt_pad, jc), dtype=np.float32)
    dl_all[:, :, : NQ * jq] = dl4.transpose(2, 0, 1)
    dl_all[:, :, NQ * jq] = np.arange(P, dtype=np.float32)[:, None]

    # dinv folded into x rows; both also shipped for the dst-side scale
    xs = np.asarray(x, np.float32) * dinv_full[:n_nodes, None]
    xT = np.zeros((hid, npad), dtype=BF16)
    xT[:, :n_nodes] = xs.T.astype(BF16)
    dinv2d = dinv_full.reshape(nt_pad, P).T.copy()  # [P, nt_pad]

    b_bcast = np.broadcast_to(np.asarray(b, np.float32), (P, out_dim)).copy()

    n_call = slot_cap  # idxs per dma_gather call (one (tile, quarter) group)
    cols_call = n_call // 16

    shared = dict(W=np.asarray(W, np.float32).astype(BF16), b_bcast=b_bcast)
    per_core = []
    for c in range(N_CORES):
        t0 = c * nt_core
        xr = np.roll(xT, -t0 * P, axis=1)
        dvc = np.ascontiguousarray(np.roll(dinv2d, -t0, axis=1)[:, :nt_core])
        dlc = np.ascontiguousarray(
            dl_all[:, t0 : t0 + nt_core, :].reshape(P, nt_core * jc).astype(BF16)
        )
        # int16 gather locals with rotated tile index; padding slots = -1
        tg_rot = (loc_tg3[t0 : t0 + nt_core] - t0) % nt_pad  # [nt_core, NQ, slot_cap]
        loc = np.where(
            valid3[t0 : t0 + nt_core] if PAD_NEG1 else True,
            loc_pp3[t0 : t0 + nt_core] * nt_pad + tg_rot,
            -1,
        ).astype(np.int16)
        # per call (= one (tile, quarter)): idx i -> [i%16, i//16]; stack calls
        # on cols; replicate x8 to fill 128 partitions
        loc_b = loc.reshape(nt_core * NQ, cols_call, 16).transpose(0, 2, 1)
        idx16 = np.tile(
            loc_b.transpose(1, 0, 2).reshape(16, nt_core * NQ * cols_call), (8, 1)
        )
        per_core.append(
            dict(
                xT=xr,
                dinv=dvc,
                dl=dlc,
                idx16=np.ascontiguousarray(idx16),
            )
        )
    meta = dict(
        n_nodes=n_nodes,
        hid=hid,
        out_dim=out_dim,
        nt_pad=nt_pad,
        npad=npad,
        nt_core=nt_core,
        jq=jq,
        jc=jc,
    )
    return meta, shared, per_core


def build_program(meta, variant="full"):
    hid, out_dim = meta["hid"], meta["out_dim"]
    nt_pad, nt_core = meta["nt_pad"], meta["nt_core"]
    jq, jc = meta["jq"], meta["jc"]
    npad = meta["npad"]
    f32, i32, i16 = mybir.dt.float32, mybir.dt.int32, mybir.dt.int16
    bf16 = mybir.dt.bfloat16

    n_call = jq * P
    cols_call = n_call // 16
    nfin = nt_core // FIN_B

    nc = bacc.Bacc(
        "TRN2",
        target_bir_lowering=False,
        debug=False,
        num_devices=N_CORES,
        num_swdge_queues=NQUEUES,
    )

    xT_d = nc.dram_tensor("xT", [hid, npad], bf16, kind="ExternalInput").ap()
    W_d = nc.dram_tensor("W", [hid, out_dim], bf16, kind="ExternalInput").ap()
    b_d = nc.dram_tensor("b_bcast", [P, out_dim], f32, kind="ExternalInput").ap()
    dinv_d = nc.dram_tensor("dinv", [P, nt_core], f32, kind="ExternalInput").ap()
    dl_d = nc.dram_tensor("dl", [P, nt_core * jc], bf16, kind="ExternalInput").ap()
    idx_d = nc.dram_tensor(
        "idx16", [P, nt_core * NQ * cols_call], i16, kind="ExternalInput"
    ).ap()
    # g rows: node n=(t*128+p) at row p*nt_pad + t (256B rows, 64 bf16 payload);
    # quarter q = rows of partitions [32q, 32q+32) — int16-addressable sub-table
    g_d = nc.dram_tensor("g", [P * nt_pad, GROW], bf16, kind="Internal").ap()
    out_d = nc.dram_tensor("out", [nt_core * P, out_dim], f32, kind="ExternalOutput").ap()

    gw3 = g_d.rearrange("(p t) d -> p t d", p=P)
    gq_d = [g_d[ts(q, 32 * nt_pad), :] for q in range(NQ)]

    do_a = variant not in ("noop",)
    do_b = variant in ("full", "nogath", "nomm", "noS")
    do_gath = variant in ("full", "nomm", "noS")
    do_smm = variant in ("full", "nogath", "noS")
    do_sbuild = variant in ("full", "nogath")

    with tile.TileContext(nc) as tc:
        with (
            tc.tile_pool(name="const", bufs=1) as const_pool,
            tc.tile_pool(name="work", bufs=3) as work,
            tc.tile_pool(name="fin", bufs=2) as fin_pool,
            tc.tile_pool(name="gath", bufs=3) as gath_pool,
            tc.tile_pool(name="smat", bufs=3) as smat_pool,
            tc.tile_pool(name="psumA", bufs=2, space="PSUM") as psumA_pool,
            tc.tile_pool(name="psumB", bufs=2, space="PSUM") as psumB_pool,
        ):
            # ---- constants ----
            W_sb = const_pool.tile([hid, out_dim], bf16)
            nc.sync.dma_start(W_sb[:], W_d[:])
            b_sb = const_pool.tile([P, out_dim], f32)
            nc.sync.dma_start(b_sb[:], b_d[:])
            dinv = const_pool.tile([P, nt_core], f32)
            nc.sync.dma_start(dinv[:], dinv_d[:])
            dl_sb = const_pool.tile([P, nt_core * jc], bf16)
            nc.sync.dma_start(dl_sb[:], dl_d[:])
            dl_v = dl_sb[:].rearrange("p (t j) -> p t j", j=jc)
            idx_sb = const_pool.tile([P, nt_core * NQ * cols_call], i16)
            nc.sync.dma_start(idx_sb[:], idx_d[:])

            iota_i = const_pool.tile([P, P], i32)
            nc.gpsimd.iota(iota_i[:], pattern=[[1, P]], base=0, channel_multiplier=0)
            iota_b = const_pool.tile([P, P], bf16)
            nc.vector.tensor_copy(iota_b[:], iota_i[:])

            # ---- pass A: g = (dinv*x) @ W in bf16 for all node tiles ----
            # own rows (this core's tiles t < nt_core) are captured into SBUF
            # on the way through for the pass-B self-loop chunk
            own_sb = const_pool.tile([P, nt_core * out_dim], bf16)
            own_v = own_sb[:].rearrange("p (t d) -> p t d", d=out_dim)
            if not do_a:
                nc.vector.memset(own_sb[:], 0.25)
            for tb in range(nt_pad // BATCH_A if do_a else 0):
                xt = work.tile([hid, BATCH_A * P], bf16, tag="xT")
                nc.sync.dma_start(xt[:], xT_d[:, ts(tb, BATCH_A * P)])
                hp = psumA_pool.tile([P, BATCH_A * out_dim], f32, tag="psA")
                for k in range(BATCH_A):
                    nc.tensor.matmul(
                        out=hp[:, ts(k, out_dim)],
                        lhsT=xt[:, ts(k, P)],
                        rhs=W_sb[:],
                        start=True,
                        stop=True,
                    )
                gt = work.tile([P, BATCH_A, out_dim], bf16, tag="gA")
                nc.scalar.activation(
                    gt[:].rearrange("p k d -> p (k d)"),
                    hp[:],
                    mybir.ActivationFunctionType.Copy,
                )
                nc.scalar.dma_start(gw3[:, ts(tb, BATCH_A), 0:out_dim], gt[:])
                lo = tb * BATCH_A
                if lo < nt_core:
                    m = min(BATCH_A, nt_core - lo)
                    nc.vector.tensor_copy(own_v[:, lo : lo + m, :], gt[:, :m, :])

            # ---- pass B ----
            if not do_b:  # timing probes: emit placeholder output stores
                zt = const_pool.tile([P, out_dim], f32)
                nc.vector.memset(zt[:], 0.5)
                for t in range(nt_core):
                    nc.scalar.dma_start(out_d[ts(t, P), :], zt[:])
            S_const = None
            if do_smm and not do_sbuild:  # "noS" probe: one shared S matrix
                S_const = const_pool.tile([P, jc * P], bf16)
                nc.vector.memset(S_const[:], 0.0078125)
            gath_const = None
            if do_b and not do_gath:  # "nomm"/"nogath" probe support
                gath_const = const_pool.tile([P, NQ * jq, GROW], bf16)
                nc.vector.memset(gath_const[:], 0.125)
            op = None
            for t in range(nt_core if do_b else 0):
                if do_gath:
                    gath = gath_pool.tile([P, NQ * jq, GROW], bf16, tag="gath")
                    for q in range(NQ):
                        nc.gpsimd.dma_gather(
                            out_ap=gath[:, ts(q, jq), :],
                            in_ap=gq_d[q][:],
                            idxs_ap=idx_sb[:, ts(t * NQ + q, cols_call)],
                            num_idxs=n_call,
                            num_idxs_reg=n_call,
                            elem_size=GROW,
                            single_packet=True,
                            queue_num=q % NQUEUES,
                        )
                else:
                    gath = gath_const
                if do_sbuild:
                    S = smat_pool.tile([P, jc * P], bf16, tag="smat")
                    nc.vector.tensor_tensor(
                        out=S[:].rearrange("p (j q) -> p j q", j=jc),
                        in0=dl_v[:, t, :, None].to_broadcast([P, jc, P]),
                        in1=iota_b[:, None, :].to_broadcast([P, jc, P]),
                        op=mybir.AluOpType.is_equal,
                    )
                else:
                    S = S_const
                k = t % FIN_B
                if do_smm:
                    if k == 0:
                        op = psumB_pool.tile([P, FIN_B, out_dim], f32, tag="psB")
                    for cc in range(jc):
                        if cc < NQ * jq:
                            q, j = divmod(cc, jq)
                            rhs = gath[:, q * jq + j, 0:out_dim]
                        else:
                            rhs = own_v[:, t, :]
                        nc.tensor.matmul(
                            out=op[:, k, :],
                            lhsT=S[:, ts(cc, P)],
                            rhs=rhs,
                            start=(cc == 0),
                            stop=(cc == jc - 1),
                        )
                if k == FIN_B - 1:
                    bb = t // FIN_B
                    if do_smm:
                        fsrc = op[:]
                    else:
                        fsrc = own_v[:, ts(bb, FIN_B), :]
                    ot = fin_pool.tile([P, FIN_B, out_dim], f32, tag="outt")
                    nc.vector.tensor_tensor(
                        out=ot[:],
                        in0=fsrc,
                        in1=dinv[:, ts(bb, FIN_B), None].to_broadcast(
                            [P, FIN_B, out_dim]
                        ),
                        op=mybir.AluOpType.mult,
                    )
                    ob = fin_pool.tile([P, FIN_B, out_dim], f32, tag="outb")
                    nc.vector.tensor_tensor(
                        out=ob[:],
                        in0=ot[:],
                        in1=b_sb[:, None, :].to_broadcast([P, FIN_B, out_dim]),
                        op=mybir.AluOpType.add,
                    )
                    osig = fin_pool.tile([P, FIN_B, out_dim], f32, tag="osig")
                    nc.scalar.activation(
                        osig[:].rearrange("p k d -> p (k d)"),
                        ob[:].rearrange("p k d -> p (k d)"),
                        mybir.ActivationFunctionType.Sigmoid,
                    )
                    nc.scalar.dma_start(
                        out_d[ts(bb, FIN_B * P), :].rearrange("(k p) d -> p k d", p=P),
                        osig[:],
                    )

    nc.compile()
    return nc


def _get_program(meta):
    key = tuple(sorted((k, v) for k, v in meta.items()))
    if key not in _prog_cache:
        _prog_cache[key] = build_program(meta)
    return _prog_cache[key]


def make_in_maps(meta, shared, per_core):
    return [dict(shared, **per_core[c]) for c in range(N_CORES)]


def kernel(x, edge_index, W, b) -> np.ndarray:
    x = np.asarray(x, np.float32)
    edge_index = np.asarray(edge_index)
    W = np.asarray(W, np.float32)
    b = np.asarray(b, np.float32)

    meta, shared, per_core = preprocess(x, edge_index, W, b)
    nc = _get_program(meta)
    in_maps = make_in_maps(meta, shared, per_core)
    res = run_bass_kernel_spmd(nc, in_maps, core_ids=list(range(N_CORES)))
    outs = [res.results[c]["out"] for c in range(N_CORES)]
    full = np.concatenate(outs, axis=0)
    return full[: meta["n_nodes"]]
